# revision 38
# baseline (speedup 1.0000x reference)
"""Distributed Trainium2 Bass kernel: 16-head causal attention with RoPE.

Problem: B=4, S=2048, D=1024, H=16 (hd=64), causal mask, interleaved RoPE
(RoFormer concatenated cos/sin cache), f32 inputs.

Sharding (8 cores): data-parallel over B (4) x tensor-parallel over head
groups (2 x 8 heads).  Core c handles batch c//2, heads (c%2)*8..(c%2)*8+7.

Per-core pipeline (bf16 compute, f32 PSUM accumulation):
  1. qT/kT (transposed, [e, s]) and v ([s, e]) projections from xT.
  2. RoPE applied in the transposed layout.  The interleaved pairing is
     de-interleaved by permuting W_q/W_k rows on the host so the rotation
     partner is a 32-partition block swap.
  3. Causal attention per head with scores kept transposed ([key, query]),
     so softmax denominators come from an extra ones-column in v (PE
     reduction) -- no partition-dim reductions needed.  exp() without
     max-subtraction (scores are O(1) for this data distribution).
  4. attn-out halves exchanged within each batch pair by AllGather (bf16,
     split in two for compute/comm overlap), then each core computes its
     512-column slice of the W_o projection.  Host concatenates.
"""

import numpy as np

B, S, D = 4, 2048, 1024
H, HD = 16, 64
HPC = 8                # heads per core
E = HPC * HD           # 512
NBLK = S // 512        # query blocks per core
NEG = -30000.0         # additive mask value (exp -> exactly 0)
RG = [[0, 1], [2, 3], [4, 5], [6, 7]]

_CACHE = {}


def _build_nc():
    import concourse.bacc as bacc
    import concourse.mybir as mybir
    import concourse.tile as tile

    dt = mybir.dt
    F32, BF = dt.float32, dt.bfloat16
    AF = mybir.ActivationFunctionType
    OP = mybir.AluOpType

    nc = bacc.Bacc("TRN2", target_bir_lowering=False, debug=False,
                   num_devices=8)

    xT = nc.dram_tensor("xT", [D, S], BF, kind="ExternalInput")
    wqT = nc.dram_tensor("wqT", [D, E], BF, kind="ExternalInput")
    wkT = nc.dram_tensor("wkT", [D, E], BF, kind="ExternalInput")
    wvT = nc.dram_tensor("wvT", [D, E], BF, kind="ExternalInput")
    woT = nc.dram_tensor("woT", [D, E], BF, kind="ExternalInput")
    cosT = nc.dram_tensor("cosT", [128, S], BF, kind="ExternalInput")
    sinT = nc.dram_tensor("sinT", [128, S], BF, kind="ExternalInput")
    mask01 = nc.dram_tensor("mask01", [2, 128, 1024], BF,
                            kind="ExternalInput")
    out = nc.dram_tensor("out", [E, S], F32, kind="ExternalOutput")

    with tile.TileContext(nc, num_cores=8) as tc, \
         tc.tile_pool(name="consts", bufs=1) as cpool, \
         tc.tile_pool(name="qkv", bufs=1) as qpool, \
         tc.tile_pool(name="attno", bufs=1) as apool, \
         tc.tile_pool(name="dram", bufs=1, space="DRAM") as dpool:

        # ---------------- constants (DMAs deferred past x block 0) ----
        cos_sb = cpool.tile([128, S], BF, name="cos_sb", tag="cos_sb")
        sin_sb = cpool.tile([128, S], BF, name="sin_sb", tag="sin_sb")
        mask_sb = []
        for d4 in range(2):
            bt = cpool.tile([128, 1024], BF, name=f"mask{d4}",
                            tag=f"mask{d4}")
            mask_sb.append(bt)

        # persistent bf16 tensors (2 heads per 128-partition tile)
        qT = [qpool.tile([128, S], BF, name=f"qT{i}", tag=f"qT{i}")
              for i in range(4)]
        kT = [qpool.tile([128, S], BF, name=f"kT{i}", tag=f"kT{i}")
              for i in range(4)]
        # v tiles [128 seq, 8 heads x (64 dims + ones column)]
        vS = [qpool.tile([128, HPC * (HD + 1)], BF, name=f"v{i}", tag=f"v{i}")
              for i in range(S // 128)]
        wq = [qpool.tile([128, E], BF, name=f"wq{c}", tag=f"wq{c}")
              for c in range(8)]
        wk = [qpool.tile([128, E], BF, name=f"wk{c}", tag=f"wk{c}")
              for c in range(8)]
        wv = [qpool.tile([128, E], BF, name=f"wv{c}", tag=f"wv{c}")
              for c in range(8)]
        attnT = [apool.tile([128, S], BF, name=f"at{i}", tag=f"at{i}")
                 for i in range(4)]

        # Per-column-block AllGather bounce buffers (internal DRAM).
        # The last block is split into two head-halves so its first
        # exchange overlaps the second half's attention.
        ag_in = [dpool.tile([E, 512], BF, name=f"ag_in{b_}", tag=f"ag_in{b_}")
                 for b_ in range(NBLK - 1)]
        ag_out = [dpool.tile([2, E, 512], BF, name=f"ag_out{b_}",
                             tag=f"ag_out{b_}") for b_ in range(NBLK - 1)]
        ag_in_l = [dpool.tile([E // 2, 512], BF, name=f"ag_inl{half}",
                              tag=f"ag_inl{half}") for half in range(2)]
        ag_out_l = [dpool.tile([2, E // 2, 512], BF, name=f"ag_outl{half}",
                               tag=f"ag_outl{half}") for half in range(2)]

        wo = [qpool.tile([128, E], BF, name=f"wo{c}", tag=f"wo{c}")
              for c in range(8)]

        # ------- projections + RoPE interleaved with attention -------
        # Block blk: project q/k/v for s-block blk, then run attention
        # query-block blk for all heads (needs k/v only up to blk).
        # Interleaving overlaps PE-heavy projections with ACT-heavy exp.
        with tc.tile_pool(name="xb", bufs=20) as xbp, \
             tc.tile_pool(name="rope", bufs=3) as rpool, \
             tc.tile_pool(name="pproj", bufs=2, space="PSUM") as pproj, \
             tc.tile_pool(name="psc", bufs=2, space="PSUM") as psc, \
             tc.tile_pool(name="pav", bufs=2, space="PSUM") as pav, \
             tc.tile_pool(name="pp", bufs=5) as ppool, \
             tc.tile_pool(name="rr", bufs=3) as rrpool, \
             tc.tile_pool(name="agsb", bufs=16) as agp, \
             tc.tile_pool(name="osb", bufs=3) as osb:
            def wo_stage(pblk):
                """Gather-loads + W_o matmuls + out DMA for column-block
                pblk (whose AllGather was issued at the end of iteration
                pblk).  Emitted one iteration later so the in-order DMA
                queue never stalls the next block's x loads behind an
                in-flight collective."""
                psl = slice(pblk * 512, (pblk + 1) * 512)
                ag_ch = []
                for ec in range(8):
                    r, m = ec // 4, ec % 4
                    gt = agp.tile([128, 512], BF, name="gt", tag="gt")
                    nc.sync.dma_start(
                        gt[:, :], ag_out[pblk][r, m * 128:(m + 1) * 128, :])
                    ag_ch.append(gt)
                for jt in range(4):
                    po = pproj.tile([128, 512], F32, name="ps", tag="ps")
                    for ec in range(8):
                        nc.tensor.matmul(
                            po[:, :],
                            wo[ec][:, jt * 128:(jt + 1) * 128],
                            ag_ch[ec][:, :],
                            start=(ec == 0), stop=(ec == 7))
                    ot = osb.tile([128, 512], F32, name="ot", tag="ot")
                    nc.vector.tensor_copy(ot[:, :], po[:, :])
                    nc.sync.dma_start(out[jt * 128:(jt + 1) * 128, psl],
                                      ot[:, :])

            xcache = {}
            A_EC, B_EC = [0, 1, 4, 5], [2, 3, 6, 7]
            ag_ch = {}

            def load_x(b_):
                sl_ = slice(b_ * 512, (b_ + 1) * 512)
                chunks = []
                for c in range(8):
                    xb = xbp.tile([128, 512], BF, name="xb", tag="xb")
                    nc.sync.dma_start(xb[:, :],
                                      xT[c * 128:(c + 1) * 128, sl_])
                    chunks.append(xb)
                xcache[b_] = chunks

            # startup order: x block 0 first, then small consts, then
            # q weights (needed first), then the rest.
            load_x(0)
            for wdram, wtiles in ((wkT, wk), (wqT, wq), (wvT, wv),
                                  (woT, wo)):
                for c in range(8):
                    nc.sync.dma_start(wtiles[c][:, :],
                                      wdram[c * 128:(c + 1) * 128, :])
                if wdram is wkT:
                    # rope tables after k weights, before q weights
                    nc.sync.dma_start(cos_sb[:, :], cosT[:, :])
                    nc.sync.dma_start(sin_sb[:, :], sinT[:, :])
                    for d4 in range(2):
                        nc.sync.dma_start(mask_sb[d4][:, :], mask01[d4])

            for blk in range(NBLK):
                sl = slice(blk * 512, (blk + 1) * 512)
                xb_chunks = xcache.pop(blk)
                if blk + 1 < NBLK:
                    load_x(blk + 1)
                # k before q within each e-tile pair so the first heads'
                # QK (needs both) unblocks as early as possible
                for et in range(4):
                    for wtiles, dstT in ((wk, kT), (wq, qT)):
                        ps = pproj.tile([128, 512], F32, name="ps", tag="ps")
                        for c in range(8):
                            nc.tensor.matmul(
                                ps[:, :],
                                wtiles[c][:, et * 128:(et + 1) * 128],
                                xb_chunks[c][:, :],
                                start=(c == 0), stop=(c == 7))
                        # RoPE in bf16 (DVE 2x mode):
                        # dst = qb*cos + swap32(qb)*sin, with the 32-row
                        # partner swap folded into the t1 muls' input APs
                        qb = rpool.tile([128, 512], BF, name="qb", tag="qb")
                        nc.vector.tensor_copy(qb[:, :], ps[:, :])
                        t1 = rpool.tile([128, 512], BF, name="t1", tag="t1")
                        # sin_sb rows are pre-swapped on the host so both
                        # inputs share a base partition; only the output
                        # lands in the partner 32-row block.
                        for a, b_ in ((0, 32), (32, 0), (64, 96), (96, 64)):
                            nc.vector.tensor_mul(t1[a:a + 32, :],
                                                 qb[b_:b_ + 32, :],
                                                 sin_sb[b_:b_ + 32, sl])
                        t2 = rpool.tile([128, 512], BF, name="t2", tag="t2")
                        nc.vector.tensor_mul(t2[:, :], qb[:, :],
                                             cos_sb[:, sl])
                        nc.vector.tensor_add(dstT[et][:, sl], t2[:, :],
                                             t1[:, :])
                for st in range(4):
                    ti = blk * 4 + st
                    psv = pproj.tile([128, 512], F32, name="ps", tag="ps")
                    for c in range(8):
                        nc.tensor.matmul(
                            psv[:, :],
                            xb_chunks[c][:, st * 128:(st + 1) * 128],
                            wv[c][:, :],
                            start=(c == 0), stop=(c == 7))
                    nc.vector.tensor_copy(
                        vS[ti][:, :].rearrange("p (h c) -> p h c",
                                               c=HD + 1)[:, :, 0:HD],
                        psv[:, :].rearrange("p (h c) -> p h c", c=HD))
                    nc.vector.memset(
                        vS[ti][:, :].rearrange("p (h c) -> p h c",
                                               c=HD + 1)[:, :, HD:HD + 1],
                        1.0)

                # ---- attention for query-block blk, all heads ----
                bi = blk
                npair = 2 * bi + 2
                isl = slice(bi * 512, (bi + 1) * 512)
                for h in range(HPC):
                    ti, off = h // 2, (h % 2) * 64
                    oa = pav.tile([65, 512], F32, name="oa", tag="oa")
                    for jp in range(npair):
                        sc = psc.tile([128, 1024], F32, name="sc", tag="sc")
                        dp = jp - 2 * bi
                        # For the outermost diagonal pair (d=2,3) the
                        # causal-valid region is only the top 256/128
                        # query columns: narrow QK/exp/mask/AV to that
                        # rectangle.  (d=0,1 stay full width so the sc
                        # tile is never read where unwritten.)
                        los = [0, 0]
                        if dp == 1:
                            los = [256, 384]
                        for half in range(2):
                            jt = 2 * jp + half
                            lo = los[half]
                            nc.tensor.matmul(
                                sc[:, half * 512 + lo:(half + 1) * 512],
                                kT[ti][off:off + 64,
                                       jt * 128:(jt + 1) * 128],
                                qT[ti][off:off + 64,
                                       bi * 512 + lo:(bi + 1) * 512],
                                start=True, stop=True)
                        pt = ppool.tile([128, 1024], BF, name="pt", tag="pt")
                        if dp == 1:
                            for half in range(2):
                                lo = half * 512 + los[half]
                                hi = (half + 1) * 512
                                nc.scalar.activation(pt[:, lo:hi],
                                                     sc[:, lo:hi], AF.Exp,
                                                     scale=0.125)
                                nc.vector.tensor_mul(pt[:, lo:hi],
                                                     pt[:, lo:hi],
                                                     mask_sb[1][:, lo:hi])
                        else:
                            nc.scalar.activation(pt[:, :], sc[:, :], AF.Exp,
                                                 scale=0.125)
                            if dp == 0:
                                nc.vector.tensor_mul(pt[:, :], pt[:, :],
                                                     mask_sb[0][:, :])
                        for half in range(2):
                            jt = 2 * jp + half
                            lo = los[half]
                            nc.tensor.matmul(
                                oa[:, lo:512],
                                vS[jt][:, h * (HD + 1):(h + 1) * (HD + 1)],
                                pt[:, half * 512 + lo:(half + 1) * 512],
                                start=(jt == 0), stop=(jt == 2 * npair - 1))
                    rc = rrpool.tile([1, 512], F32, name="rc", tag="rc")
                    nc.vector.reciprocal(rc[:, :], oa[64:65, :])
                    bcb = rrpool.tile([64, 512], F32, name="bcb", tag="bcb")
                    nc.gpsimd.partition_broadcast(bcb[:, :], rc[0:1, :])
                    nc.vector.scalar_tensor_tensor(
                        attnT[ti][off:off + 64, isl], oa[0:64, :], 1.0,
                        bcb[:, :], OP.mult, OP.mult)
                    # last block: exchange each head-half as soon as its
                    # attention completes, so only the second (quarter-
                    # size) AllGather is exposed at the end.
                    if blk == NBLK - 1 and h in (3, 7):
                        half = h // 4
                        for ti2 in range(2):
                            nc.sync.dma_start(
                                ag_in_l[half][ti2 * 128:(ti2 + 1) * 128, :],
                                attnT[half * 2 + ti2][:, isl])
                        nc.gpsimd.collective_compute(
                            "AllGather", OP.bypass, replica_groups=RG,
                            ins=[ag_in_l[half][:, :].opt()],
                            outs=[ag_out_l[half][:, :, :].opt()])
                        if half == 0:
                            # W_o for blocks 1/2 (collectives long done):
                            # their DMA loads must precede the second
                            # half's ag_in on the in-order queue, and
                            # their matmuls fill PE alongside heads 4-7.
                            wo_stage(1)
                            wo_stage(2)
                            # final-block A-half gather loads early too
                            for ec in A_EC:
                                r, m = ec // 4, ec % 4
                                srcb = ag_out_l[m // 2]
                                gt = agp.tile([128, 512], BF, name="gt",
                                              tag="gt")
                                nc.sync.dma_start(
                                    gt[:, :],
                                    srcb[r, (m % 2) * 128:
                                         (m % 2 + 1) * 128, :])
                                ag_ch[ec] = gt

                if blk < NBLK - 1:
                    # AllGather for column-block blk; its W_o stage is
                    # emitted at the end of the next iteration.
                    for ti4 in range(4):
                        nc.sync.dma_start(
                            ag_in[blk][ti4 * 128:(ti4 + 1) * 128, :],
                            attnT[ti4][:, isl])
                    nc.gpsimd.collective_compute(
                        "AllGather", OP.bypass, replica_groups=RG,
                        ins=[ag_in[blk][:, :].opt()],
                        outs=[ag_out[blk][:, :, :].opt()])
                # Deferred W_o stages; blocks 1/2 are emitted after the
                # final AllGathers are issued, to fill PE while they fly.
                for p_ in {2: (0,)}.get(blk, ()):
                    wo_stage(p_)

            # Final block's W_o, A/B-half software-pipelined: A halves
            # (loads emitted at h==3) need only the first AllGather; the
            # exposed wait on the last AllGather shrinks to the B halves.
            psl = slice((NBLK - 1) * 512, NBLK * 512)
            for ec in B_EC:
                r, m = ec // 4, ec % 4
                srcb = ag_out_l[m // 2]
                gt = agp.tile([128, 512], BF, name="gt", tag="gt")
                nc.sync.dma_start(
                    gt[:, :], srcb[r, (m % 2) * 128:(m % 2 + 1) * 128, :])
                ag_ch[ec] = gt
            po_t = {}

            def wo_half(jt, ecs, start, stop):
                if jt not in po_t:
                    po_t[jt] = pproj.tile([128, 512], F32, name="ps",
                                          tag="ps")
                for idx, ec in enumerate(ecs):
                    nc.tensor.matmul(
                        po_t[jt][:, :],
                        wo[ec][:, jt * 128:(jt + 1) * 128],
                        ag_ch[ec][:, :],
                        start=start and idx == 0,
                        stop=stop and idx == len(ecs) - 1)
                if stop:
                    ot = osb.tile([128, 512], F32, name="ot", tag="ot")
                    nc.vector.tensor_copy(ot[:, :], po_t[jt][:, :])
                    nc.sync.dma_start(out[jt * 128:(jt + 1) * 128, psl],
                                      ot[:, :])

            wo_half(0, A_EC, True, False)
            wo_half(1, A_EC, True, False)
            wo_half(0, B_EC, False, True)
            wo_half(2, A_EC, True, False)
            wo_half(1, B_EC, False, True)
            wo_half(3, A_EC, True, False)
            wo_half(2, B_EC, False, True)
            wo_half(3, B_EC, False, True)
    nc.finalize()
    return nc


def _host_prep(x, W_q, W_k, W_v, W_o, mask):
    causal = np.triu(np.ones((S, S), dtype=bool), k=1)
    m = np.asarray(mask)
    assert m.shape == (B, S, S) and all(
        np.array_equal(m[b], causal) for b in range(B)), \
        "kernel is specialized for the causal mask"

    perm = np.concatenate([np.arange(0, HD, 2), np.arange(1, HD, 2)])
    permD = (np.arange(H)[:, None] * HD + perm[None, :]).reshape(-1)
    Wq_p = np.asarray(W_q)[permD]
    Wk_p = np.asarray(W_k)[permD]

    inv = 1.0 / (10000.0 ** (np.arange(0, HD, 2, dtype=np.float64) / HD))
    t = np.arange(S, dtype=np.float64)
    emb = np.concatenate([t[:, None] * inv[None, :]] * 2, axis=1)  # [S, 64]
    cosF = np.cos(emb).T[perm]                       # [64, S]
    sinF = np.sin(emb).T[perm]
    sgn = np.concatenate([-np.ones(32), np.ones(32)])[:, None]
    import ml_dtypes
    bf16 = ml_dtypes.bfloat16
    cos128 = np.ascontiguousarray(np.tile(cosF, (2, 1)).astype(bf16))
    sin128 = np.tile(sinF * sgn, (2, 1))
    swap = np.concatenate([np.arange(32, 64), np.arange(0, 32),
                           np.arange(96, 128), np.arange(64, 96)])
    sin128 = np.ascontiguousarray(sin128[swap].astype(bf16))

    r = np.arange(128)[:, None]
    c = np.arange(512)[None, :]
    b4 = [np.where(d4 * 128 + r > c, 0.0, 1.0).astype(bf16)
          for d4 in range(4)]
    mask_np = np.stack([np.concatenate([b4[0], b4[1]], axis=1),
                        np.concatenate([b4[2], b4[3]], axis=1)])

    in_maps = []
    for core in range(8):
        b, hg = core // 2, core % 2
        rs = slice(hg * E, (hg + 1) * E)
        in_maps.append({
            "xT": np.ascontiguousarray(np.asarray(x)[b].T.astype(bf16)),
            "wqT": np.ascontiguousarray(Wq_p[rs].T.astype(bf16)),
            "wkT": np.ascontiguousarray(Wk_p[rs].T.astype(bf16)),
            "wvT": np.ascontiguousarray(np.asarray(W_v)[rs].T.astype(bf16)),
            "woT": np.ascontiguousarray(np.asarray(W_o)[rs].T.astype(bf16)),
            "cosT": cos128,
            "sinT": sin128,
            "mask01": mask_np,
        })
    return in_maps


def kernel(x, W_q, W_k, W_v, W_o, mask, _trace=False):
    from concourse.bass_utils import run_bass_kernel_spmd

    if "nc" not in _CACHE:
        _CACHE["nc"] = _build_nc()
    nc = _CACHE["nc"]
    in_maps = _host_prep(x, W_q, W_k, W_v, W_o, mask)
    res = run_bass_kernel_spmd(nc, in_maps, core_ids=list(range(8)),
                               trace=_trace)
    _CACHE["last_result"] = res
    full = np.empty((B, S, D), dtype=np.float32)
    for core in range(8):
        b, hg = core // 2, core % 2
        full[b, :, hg * E:(hg + 1) * E] = res.results[core]["out"].T
    return full


# revision 42
# speedup vs baseline: 1.0014x; 1.0014x over previous
"""Distributed Trainium2 Bass kernel: 16-head causal attention with RoPE.

Problem: B=4, S=2048, D=1024, H=16 (hd=64), causal mask, interleaved RoPE
(RoFormer concatenated cos/sin cache), f32 inputs.

Sharding (8 cores): data-parallel over B (4) x tensor-parallel over head
groups (2 x 8 heads).  Core c handles batch c//2, heads (c%2)*8..(c%2)*8+7.

Per-core pipeline (bf16 compute, f32 PSUM accumulation):
  1. qT/kT (transposed, [e, s]) and v ([s, e]) projections from xT.
  2. RoPE applied in the transposed layout.  The interleaved pairing is
     de-interleaved by permuting W_q/W_k rows on the host so the rotation
     partner is a 32-partition block swap.
  3. Causal attention per head with scores kept transposed ([key, query]),
     so softmax denominators come from an extra ones-column in v (PE
     reduction) -- no partition-dim reductions needed.  exp() without
     max-subtraction (scores are O(1) for this data distribution).
  4. attn-out halves exchanged within each batch pair by AllGather (bf16,
     split in two for compute/comm overlap), then each core computes its
     512-column slice of the W_o projection.  Host concatenates.
"""

import numpy as np

B, S, D = 4, 2048, 1024
H, HD = 16, 64
HPC = 8                # heads per core
E = HPC * HD           # 512
NBLK = S // 512        # query blocks per core
NEG = -30000.0         # additive mask value (exp -> exactly 0)
RG = [[0, 1], [2, 3], [4, 5], [6, 7]]

_CACHE = {}


def _build_nc():
    import concourse.bacc as bacc
    import concourse.mybir as mybir
    import concourse.tile as tile

    dt = mybir.dt
    F32, BF = dt.float32, dt.bfloat16
    AF = mybir.ActivationFunctionType
    OP = mybir.AluOpType

    nc = bacc.Bacc("TRN2", target_bir_lowering=False, debug=False,
                   num_devices=8)

    xT = nc.dram_tensor("xT", [D, S], BF, kind="ExternalInput")
    wqT = nc.dram_tensor("wqT", [D, E], BF, kind="ExternalInput")
    wkT = nc.dram_tensor("wkT", [D, E], BF, kind="ExternalInput")
    wvT = nc.dram_tensor("wvT", [D, E], BF, kind="ExternalInput")
    woT = nc.dram_tensor("woT", [D, E], BF, kind="ExternalInput")
    cosT = nc.dram_tensor("cosT", [128, S], BF, kind="ExternalInput")
    sinT = nc.dram_tensor("sinT", [128, S], BF, kind="ExternalInput")
    mask01 = nc.dram_tensor("mask01", [2, 128, 1024], BF,
                            kind="ExternalInput")
    out = nc.dram_tensor("out", [E, S], F32, kind="ExternalOutput")

    with tile.TileContext(nc, num_cores=8) as tc, \
         tc.tile_pool(name="consts", bufs=1) as cpool, \
         tc.tile_pool(name="qkv", bufs=1) as qpool, \
         tc.tile_pool(name="attno", bufs=1) as apool, \
         tc.tile_pool(name="dram", bufs=1, space="DRAM") as dpool:

        # ---------------- constants (DMAs deferred past x block 0) ----
        cos_sb = cpool.tile([128, S], BF, name="cos_sb", tag="cos_sb")
        sin_sb = cpool.tile([128, S], BF, name="sin_sb", tag="sin_sb")
        mask_sb = []
        for d4 in range(2):
            bt = cpool.tile([128, 1024], BF, name=f"mask{d4}",
                            tag=f"mask{d4}")
            mask_sb.append(bt)

        # persistent bf16 tensors (2 heads per 128-partition tile)
        qT = [qpool.tile([128, S], BF, name=f"qT{i}", tag=f"qT{i}")
              for i in range(4)]
        kT = [qpool.tile([128, S], BF, name=f"kT{i}", tag=f"kT{i}")
              for i in range(4)]
        # v tiles [128 seq, 8 heads x (64 dims + ones column)]
        vS = [qpool.tile([128, HPC * (HD + 1)], BF, name=f"v{i}", tag=f"v{i}")
              for i in range(S // 128)]
        wq = [qpool.tile([128, E], BF, name=f"wq{c}", tag=f"wq{c}")
              for c in range(8)]
        wk = [qpool.tile([128, E], BF, name=f"wk{c}", tag=f"wk{c}")
              for c in range(8)]
        wv = [qpool.tile([128, E], BF, name=f"wv{c}", tag=f"wv{c}")
              for c in range(8)]
        attnT = [apool.tile([128, S], BF, name=f"at{i}", tag=f"at{i}")
                 for i in range(4)]

        # Per-column-block AllGather bounce buffers (internal DRAM).
        # The last block is split into two head-halves so its first
        # exchange overlaps the second half's attention.
        ag_in = [dpool.tile([E, 512], BF, name=f"ag_in{b_}", tag=f"ag_in{b_}")
                 for b_ in range(NBLK - 1)]
        ag_out = [dpool.tile([2, E, 512], BF, name=f"ag_out{b_}",
                             tag=f"ag_out{b_}") for b_ in range(NBLK - 1)]
        ag_in_l = [dpool.tile([E // 2, 512], BF, name=f"ag_inl{half}",
                              tag=f"ag_inl{half}") for half in range(2)]
        ag_out_l = [dpool.tile([2, E // 2, 512], BF, name=f"ag_outl{half}",
                               tag=f"ag_outl{half}") for half in range(2)]

        wo = [qpool.tile([128, E], BF, name=f"wo{c}", tag=f"wo{c}")
              for c in range(8)]

        # ------- projections + RoPE interleaved with attention -------
        # Block blk: project q/k/v for s-block blk, then run attention
        # query-block blk for all heads (needs k/v only up to blk).
        # Interleaving overlaps PE-heavy projections with ACT-heavy exp.
        with tc.tile_pool(name="xb", bufs=20) as xbp, \
             tc.tile_pool(name="rope", bufs=3) as rpool, \
             tc.tile_pool(name="pproj", bufs=2, space="PSUM") as pproj, \
             tc.tile_pool(name="psc", bufs=2, space="PSUM") as psc, \
             tc.tile_pool(name="pav", bufs=2, space="PSUM") as pav, \
             tc.tile_pool(name="pp", bufs=5) as ppool, \
             tc.tile_pool(name="rr", bufs=3) as rrpool, \
             tc.tile_pool(name="agsb", bufs=16) as agp, \
             tc.tile_pool(name="osb", bufs=3) as osb:
            def wo_stage(pblk):
                """Gather-loads + W_o matmuls + out DMA for column-block
                pblk (whose AllGather was issued at the end of iteration
                pblk).  Emitted one iteration later so the in-order DMA
                queue never stalls the next block's x loads behind an
                in-flight collective."""
                psl = slice(pblk * 512, (pblk + 1) * 512)
                ch = wo_loaded.pop(pblk, None)
                if ch is None:
                    ch = wo_loads(pblk)
                for jt in range(4):
                    po = pproj.tile([128, 512], F32, name="ps", tag="ps")
                    for ec in range(8):
                        nc.tensor.matmul(
                            po[:, :],
                            wo[ec][:, jt * 128:(jt + 1) * 128],
                            ch[ec][:, :],
                            start=(ec == 0), stop=(ec == 7))
                    ot = osb.tile([128, 512], F32, name="ot", tag="ot")
                    nc.vector.tensor_copy(ot[:, :], po[:, :])
                    nc.sync.dma_start(out[jt * 128:(jt + 1) * 128, psl],
                                      ot[:, :])

            def wo_loads(pblk):
                ch = []
                for ec in range(8):
                    r, m = ec // 4, ec % 4
                    gt = agp.tile([128, 512], BF, name="gt", tag="gt")
                    nc.sync.dma_start(
                        gt[:, :], ag_out[pblk][r, m * 128:(m + 1) * 128, :])
                    ch.append(gt)
                return ch

            xcache = {}
            A_EC, B_EC = [0, 1, 4, 5], [2, 3, 6, 7]
            ag_ch = {}
            wo_loaded = {}

            def load_x(b_):
                sl_ = slice(b_ * 512, (b_ + 1) * 512)
                chunks = []
                for c in range(8):
                    xb = xbp.tile([128, 512], BF, name="xb", tag="xb")
                    nc.sync.dma_start(xb[:, :],
                                      xT[c * 128:(c + 1) * 128, sl_])
                    chunks.append(xb)
                xcache[b_] = chunks

            # startup order: x block 0 first, then small consts, then
            # q weights (needed first), then the rest.
            load_x(0)
            for wdram, wtiles in ((wkT, wk), (wqT, wq), (wvT, wv),
                                  (woT, wo)):
                for c in range(8):
                    nc.sync.dma_start(wtiles[c][:, :],
                                      wdram[c * 128:(c + 1) * 128, :])
                if wdram is wkT:
                    # rope tables after k weights, before q weights
                    nc.sync.dma_start(cos_sb[:, :], cosT[:, :])
                    nc.sync.dma_start(sin_sb[:, :], sinT[:, :])
                    for d4 in range(2):
                        nc.sync.dma_start(mask_sb[d4][:, :], mask01[d4])

            for blk in range(NBLK):
                sl = slice(blk * 512, (blk + 1) * 512)
                xb_chunks = xcache.pop(blk)
                if blk + 1 < NBLK:
                    load_x(blk + 1)
                # k before q within each e-tile pair so the first heads'
                # QK (needs both) unblocks as early as possible
                for et in range(4):
                    for wtiles, dstT in ((wk, kT), (wq, qT)):
                        ps = pproj.tile([128, 512], F32, name="ps", tag="ps")
                        for c in range(8):
                            nc.tensor.matmul(
                                ps[:, :],
                                wtiles[c][:, et * 128:(et + 1) * 128],
                                xb_chunks[c][:, :],
                                start=(c == 0), stop=(c == 7))
                        # RoPE in bf16 (DVE 2x mode):
                        # dst = qb*cos + swap32(qb)*sin, with the 32-row
                        # partner swap folded into the t1 muls' input APs
                        qb = rpool.tile([128, 512], BF, name="qb", tag="qb")
                        nc.vector.tensor_copy(qb[:, :], ps[:, :])
                        t1 = rpool.tile([128, 512], BF, name="t1", tag="t1")
                        # sin_sb rows are pre-swapped on the host so both
                        # inputs share a base partition; only the output
                        # lands in the partner 32-row block.
                        for a, b_ in ((0, 32), (32, 0), (64, 96), (96, 64)):
                            nc.vector.tensor_mul(t1[a:a + 32, :],
                                                 qb[b_:b_ + 32, :],
                                                 sin_sb[b_:b_ + 32, sl])
                        t2 = rpool.tile([128, 512], BF, name="t2", tag="t2")
                        nc.vector.tensor_mul(t2[:, :], qb[:, :],
                                             cos_sb[:, sl])
                        nc.vector.tensor_add(dstT[et][:, sl], t2[:, :],
                                             t1[:, :])
                for st in range(4):
                    ti = blk * 4 + st
                    psv = pproj.tile([128, 512], F32, name="ps", tag="ps")
                    for c in range(8):
                        nc.tensor.matmul(
                            psv[:, :],
                            xb_chunks[c][:, st * 128:(st + 1) * 128],
                            wv[c][:, :],
                            start=(c == 0), stop=(c == 7))
                    nc.vector.tensor_copy(
                        vS[ti][:, :].rearrange("p (h c) -> p h c",
                                               c=HD + 1)[:, :, 0:HD],
                        psv[:, :].rearrange("p (h c) -> p h c", c=HD))
                    nc.vector.memset(
                        vS[ti][:, :].rearrange("p (h c) -> p h c",
                                               c=HD + 1)[:, :, HD:HD + 1],
                        1.0)

                # ---- attention for query-block blk, all heads ----
                bi = blk
                npair = 2 * bi + 2
                isl = slice(bi * 512, (bi + 1) * 512)
                for h in range(HPC):
                    ti, off = h // 2, (h % 2) * 64
                    oa = pav.tile([65, 512], F32, name="oa", tag="oa")
                    for jp in range(npair):
                        sc = psc.tile([128, 1024], F32, name="sc", tag="sc")
                        dp = jp - 2 * bi
                        # For the outermost diagonal pair (d=2,3) the
                        # causal-valid region is only the top 256/128
                        # query columns: narrow QK/exp/mask/AV to that
                        # rectangle.  (d=0,1 stay full width so the sc
                        # tile is never read where unwritten.)
                        los = [0, 0]
                        if dp == 1:
                            los = [256, 384]
                        for half in range(2):
                            jt = 2 * jp + half
                            lo = los[half]
                            nc.tensor.matmul(
                                sc[:, half * 512 + lo:(half + 1) * 512],
                                kT[ti][off:off + 64,
                                       jt * 128:(jt + 1) * 128],
                                qT[ti][off:off + 64,
                                       bi * 512 + lo:(bi + 1) * 512],
                                start=True, stop=True)
                        pt = ppool.tile([128, 1024], BF, name="pt", tag="pt")
                        if dp == 1:
                            for half in range(2):
                                lo = half * 512 + los[half]
                                hi = (half + 1) * 512
                                nc.scalar.activation(pt[:, lo:hi],
                                                     sc[:, lo:hi], AF.Exp,
                                                     scale=0.125)
                                nc.vector.tensor_mul(pt[:, lo:hi],
                                                     pt[:, lo:hi],
                                                     mask_sb[1][:, lo:hi])
                        else:
                            nc.scalar.activation(pt[:, :], sc[:, :], AF.Exp,
                                                 scale=0.125)
                            if dp == 0:
                                nc.vector.tensor_mul(pt[:, :], pt[:, :],
                                                     mask_sb[0][:, :])
                        for half in range(2):
                            jt = 2 * jp + half
                            lo = los[half]
                            nc.tensor.matmul(
                                oa[:, lo:512],
                                vS[jt][:, h * (HD + 1):(h + 1) * (HD + 1)],
                                pt[:, half * 512 + lo:(half + 1) * 512],
                                start=(jt == 0), stop=(jt == 2 * npair - 1))
                    rc = rrpool.tile([1, 512], F32, name="rc", tag="rc")
                    nc.vector.reciprocal(rc[:, :], oa[64:65, :])
                    bcb = rrpool.tile([64, 512], F32, name="bcb", tag="bcb")
                    nc.gpsimd.partition_broadcast(bcb[:, :], rc[0:1, :])
                    nc.vector.scalar_tensor_tensor(
                        attnT[ti][off:off + 64, isl], oa[0:64, :], 1.0,
                        bcb[:, :], OP.mult, OP.mult)
                    # last block: exchange each head-half as soon as its
                    # attention completes, so only the second (quarter-
                    # size) AllGather is exposed at the end.
                    if blk == NBLK - 1 and h in (3, 7):
                        half = h // 4
                        for ti2 in range(2):
                            nc.sync.dma_start(
                                ag_in_l[half][ti2 * 128:(ti2 + 1) * 128, :],
                                attnT[half * 2 + ti2][:, isl])
                        nc.gpsimd.collective_compute(
                            "AllGather", OP.bypass, replica_groups=RG,
                            ins=[ag_in_l[half][:, :].opt()],
                            outs=[ag_out_l[half][:, :, :].opt()])
                        if half == 0:
                            # W_o for blocks 1/2 (collectives long done):
                            # their DMA loads must precede the second
                            # half's ag_in on the in-order queue, and
                            # (matmuls are emitted after the attention
                            # loop so the in-order PE queue reaches them
                            # during the final AllGather's flight)
                            for p_ in range(3):
                                wo_loaded[p_] = wo_loads(p_)
                            # final-block A-half gather loads early too
                            for ec in A_EC:
                                r, m = ec // 4, ec % 4
                                srcb = ag_out_l[m // 2]
                                gt = agp.tile([128, 512], BF, name="gt",
                                              tag="gt")
                                nc.sync.dma_start(
                                    gt[:, :],
                                    srcb[r, (m % 2) * 128:
                                         (m % 2 + 1) * 128, :])
                                ag_ch[ec] = gt

                if blk < NBLK - 1:
                    # AllGather for column-block blk; its W_o stage is
                    # emitted at the end of the next iteration.
                    for ti4 in range(4):
                        nc.sync.dma_start(
                            ag_in[blk][ti4 * 128:(ti4 + 1) * 128, :],
                            attnT[ti4][:, isl])
                    nc.gpsimd.collective_compute(
                        "AllGather", OP.bypass, replica_groups=RG,
                        ins=[ag_in[blk][:, :].opt()],
                        outs=[ag_out[blk][:, :, :].opt()])



            # W_o matmuls for blocks 0-2: PE reaches these right after
            # the last attention head, covering the final AllGather.
            wo_stage(0)
            wo_stage(1)
            wo_stage(2)

            # Final block's W_o, A/B-half software-pipelined: A halves
            # (loads emitted at h==3) need only the first AllGather; the
            # exposed wait on the last AllGather shrinks to the B halves.
            psl = slice((NBLK - 1) * 512, NBLK * 512)
            for ec in B_EC:
                r, m = ec // 4, ec % 4
                srcb = ag_out_l[m // 2]
                gt = agp.tile([128, 512], BF, name="gt", tag="gt")
                nc.sync.dma_start(
                    gt[:, :], srcb[r, (m % 2) * 128:(m % 2 + 1) * 128, :])
                ag_ch[ec] = gt
            po_t = {}

            def wo_half(jt, ecs, start, stop):
                if jt not in po_t:
                    po_t[jt] = pproj.tile([128, 512], F32, name="ps",
                                          tag="ps")
                for idx, ec in enumerate(ecs):
                    nc.tensor.matmul(
                        po_t[jt][:, :],
                        wo[ec][:, jt * 128:(jt + 1) * 128],
                        ag_ch[ec][:, :],
                        start=start and idx == 0,
                        stop=stop and idx == len(ecs) - 1)
                if stop:
                    ot = osb.tile([128, 512], F32, name="ot", tag="ot")
                    nc.vector.tensor_copy(ot[:, :], po_t[jt][:, :])
                    nc.sync.dma_start(out[jt * 128:(jt + 1) * 128, psl],
                                      ot[:, :])

            wo_half(0, A_EC, True, False)
            wo_half(1, A_EC, True, False)
            wo_half(0, B_EC, False, True)
            wo_half(2, A_EC, True, False)
            wo_half(1, B_EC, False, True)
            wo_half(3, A_EC, True, False)
            wo_half(2, B_EC, False, True)
            wo_half(3, B_EC, False, True)
    nc.finalize()
    return nc


def _host_prep(x, W_q, W_k, W_v, W_o, mask):
    causal = np.triu(np.ones((S, S), dtype=bool), k=1)
    m = np.asarray(mask)
    assert m.shape == (B, S, S) and all(
        np.array_equal(m[b], causal) for b in range(B)), \
        "kernel is specialized for the causal mask"

    perm = np.concatenate([np.arange(0, HD, 2), np.arange(1, HD, 2)])
    permD = (np.arange(H)[:, None] * HD + perm[None, :]).reshape(-1)
    Wq_p = np.asarray(W_q)[permD]
    Wk_p = np.asarray(W_k)[permD]

    inv = 1.0 / (10000.0 ** (np.arange(0, HD, 2, dtype=np.float64) / HD))
    t = np.arange(S, dtype=np.float64)
    emb = np.concatenate([t[:, None] * inv[None, :]] * 2, axis=1)  # [S, 64]
    cosF = np.cos(emb).T[perm]                       # [64, S]
    sinF = np.sin(emb).T[perm]
    sgn = np.concatenate([-np.ones(32), np.ones(32)])[:, None]
    import ml_dtypes
    bf16 = ml_dtypes.bfloat16
    cos128 = np.ascontiguousarray(np.tile(cosF, (2, 1)).astype(bf16))
    sin128 = np.tile(sinF * sgn, (2, 1))
    swap = np.concatenate([np.arange(32, 64), np.arange(0, 32),
                           np.arange(96, 128), np.arange(64, 96)])
    sin128 = np.ascontiguousarray(sin128[swap].astype(bf16))

    r = np.arange(128)[:, None]
    c = np.arange(512)[None, :]
    b4 = [np.where(d4 * 128 + r > c, 0.0, 1.0).astype(bf16)
          for d4 in range(4)]
    mask_np = np.stack([np.concatenate([b4[0], b4[1]], axis=1),
                        np.concatenate([b4[2], b4[3]], axis=1)])

    in_maps = []
    for core in range(8):
        b, hg = core // 2, core % 2
        rs = slice(hg * E, (hg + 1) * E)
        in_maps.append({
            "xT": np.ascontiguousarray(np.asarray(x)[b].T.astype(bf16)),
            "wqT": np.ascontiguousarray(Wq_p[rs].T.astype(bf16)),
            "wkT": np.ascontiguousarray(Wk_p[rs].T.astype(bf16)),
            "wvT": np.ascontiguousarray(np.asarray(W_v)[rs].T.astype(bf16)),
            "woT": np.ascontiguousarray(np.asarray(W_o)[rs].T.astype(bf16)),
            "cosT": cos128,
            "sinT": sin128,
            "mask01": mask_np,
        })
    return in_maps


def kernel(x, W_q, W_k, W_v, W_o, mask, _trace=False):
    from concourse.bass_utils import run_bass_kernel_spmd

    if "nc" not in _CACHE:
        _CACHE["nc"] = _build_nc()
    nc = _CACHE["nc"]
    in_maps = _host_prep(x, W_q, W_k, W_v, W_o, mask)
    res = run_bass_kernel_spmd(nc, in_maps, core_ids=list(range(8)),
                               trace=_trace)
    _CACHE["last_result"] = res
    full = np.empty((B, S, D), dtype=np.float32)
    for core in range(8):
        b, hg = core // 2, core % 2
        full[b, :, hg * E:(hg + 1) * E] = res.results[core]["out"].T
    return full


# revision 47
# speedup vs baseline: 1.1036x; 1.1021x over previous
"""Distributed Trainium2 Bass kernel: 16-head causal attention with RoPE.

Problem: B=4, S=2048, D=1024, H=16 (hd=64), causal mask, interleaved RoPE
(RoFormer concatenated cos/sin cache), f32 inputs.

Sharding (8 cores): data-parallel over B (4) x tensor-parallel over head
groups (2 x 8 heads).  Core c handles batch c//2, heads (c%2)*8..(c%2)*8+7.

Per-core pipeline (bf16 compute, f32 PSUM accumulation):
  1. qT/kT (transposed, [e, s]) and v ([s, e]) projections from xT.
  2. RoPE applied in the transposed layout.  The interleaved pairing is
     de-interleaved by permuting W_q/W_k rows on the host so the rotation
     partner is a 32-partition block swap.
  3. Causal attention per head with scores kept transposed ([key, query]),
     so softmax denominators come from an extra ones-column in v (PE
     reduction) -- no partition-dim reductions needed.  exp() without
     max-subtraction (scores are O(1) for this data distribution).
  4. attn-out halves exchanged within each batch pair by AllGather (bf16,
     split in two for compute/comm overlap), then each core computes its
     512-column slice of the W_o projection.  Host concatenates.
"""

import numpy as np

B, S, D = 4, 2048, 1024
H, HD = 16, 64
HPC = 8                # heads per core
E = HPC * HD           # 512
NBLK = S // 512        # query blocks per core
NEG = -30000.0         # additive mask value (exp -> exactly 0)
RG = [[0, 1], [2, 3], [4, 5], [6, 7]]

_CACHE = {}


def _build_nc():
    import concourse.bacc as bacc
    import concourse.mybir as mybir
    import concourse.tile as tile

    dt = mybir.dt
    F32, BF = dt.float32, dt.bfloat16
    AF = mybir.ActivationFunctionType
    OP = mybir.AluOpType

    nc = bacc.Bacc("TRN2", target_bir_lowering=False, debug=False,
                   num_devices=8)

    xT = nc.dram_tensor("xT", [D, S], BF, kind="ExternalInput")
    wqT = nc.dram_tensor("wqT", [D, E], BF, kind="ExternalInput")
    wkT = nc.dram_tensor("wkT", [D, E], BF, kind="ExternalInput")
    wvT = nc.dram_tensor("wvT", [D, E], BF, kind="ExternalInput")
    woT = nc.dram_tensor("woT", [D, E], BF, kind="ExternalInput")
    cosT = nc.dram_tensor("cosT", [128, S], BF, kind="ExternalInput")
    sinT = nc.dram_tensor("sinT", [128, S], BF, kind="ExternalInput")
    mask01 = nc.dram_tensor("mask01", [2, 128, 1024], BF,
                            kind="ExternalInput")
    out = nc.dram_tensor("out", [E, S], F32, kind="ExternalOutput")

    with tile.TileContext(nc, num_cores=8) as tc, \
         tc.tile_pool(name="consts", bufs=1) as cpool, \
         tc.tile_pool(name="qkv", bufs=1) as qpool, \
         tc.tile_pool(name="attno", bufs=1) as apool, \
         tc.tile_pool(name="dram", bufs=1, space="DRAM") as dpool:

        # ---------------- constants (DMAs deferred past x block 0) ----
        cos_sb = cpool.tile([128, S], BF, name="cos_sb", tag="cos_sb")
        sin_sb = cpool.tile([128, S], BF, name="sin_sb", tag="sin_sb")
        mask_sb = []
        for d4 in range(2):
            bt = cpool.tile([128, 1024], BF, name=f"mask{d4}",
                            tag=f"mask{d4}")
            mask_sb.append(bt)

        # persistent bf16 tensors (2 heads per 128-partition tile)
        qT = [qpool.tile([128, S], BF, name=f"qT{i}", tag=f"qT{i}")
              for i in range(4)]
        kT = [qpool.tile([128, S], BF, name=f"kT{i}", tag=f"kT{i}")
              for i in range(4)]
        # v tiles [128 seq, 8 heads x (64 dims + ones column)]
        vS = [qpool.tile([128, HPC * (HD + 1)], BF, name=f"v{i}", tag=f"v{i}")
              for i in range(S // 128)]
        wq = [qpool.tile([128, E], BF, name=f"wq{c}", tag=f"wq{c}")
              for c in range(8)]
        wk = [qpool.tile([128, E], BF, name=f"wk{c}", tag=f"wk{c}")
              for c in range(8)]
        wv = [qpool.tile([128, E], BF, name=f"wv{c}", tag=f"wv{c}")
              for c in range(8)]
        attnT = [apool.tile([128, S], BF, name=f"at{i}", tag=f"at{i}")
                 for i in range(4)]

        # Per-column-block AllGather bounce buffers (internal DRAM).
        # The last block is split into two head-halves so its first
        # exchange overlaps the second half's attention.
        ag_in = [dpool.tile([E, 512], BF, name=f"ag_in{b_}", tag=f"ag_in{b_}")
                 for b_ in range(NBLK - 1)]
        ag_out = [dpool.tile([2, E, 512], BF, name=f"ag_out{b_}",
                             tag=f"ag_out{b_}") for b_ in range(NBLK - 1)]
        ag_in_l = [dpool.tile([E // 2, 512], BF, name=f"ag_inl{half}",
                              tag=f"ag_inl{half}") for half in range(2)]
        ag_out_l = [dpool.tile([2, E // 2, 512], BF, name=f"ag_outl{half}",
                               tag=f"ag_outl{half}") for half in range(2)]

        wo = [qpool.tile([128, E], BF, name=f"wo{c}", tag=f"wo{c}")
              for c in range(8)]

        # ------- projections + RoPE interleaved with attention -------
        # Block blk: project q/k/v for s-block blk, then run attention
        # query-block blk for all heads (needs k/v only up to blk).
        # Interleaving overlaps PE-heavy projections with ACT-heavy exp.
        with tc.tile_pool(name="xb", bufs=20) as xbp, \
             tc.tile_pool(name="rope", bufs=3) as rpool, \
             tc.tile_pool(name="pproj", bufs=2, space="PSUM") as pproj, \
             tc.tile_pool(name="psc", bufs=2, space="PSUM") as psc, \
             tc.tile_pool(name="pav", bufs=2, space="PSUM") as pav, \
             tc.tile_pool(name="pp", bufs=5) as ppool, \
             tc.tile_pool(name="rr", bufs=5) as rrpool, \
             tc.tile_pool(name="agsb", bufs=16) as agp, \
             tc.tile_pool(name="osb", bufs=3) as osb:
            def wo_stage(pblk):
                """Gather-loads + W_o matmuls + out DMA for column-block
                pblk (whose AllGather was issued at the end of iteration
                pblk).  Emitted one iteration later so the in-order DMA
                queue never stalls the next block's x loads behind an
                in-flight collective."""
                psl = slice(pblk * 512, (pblk + 1) * 512)
                ch = wo_loaded.pop(pblk, None)
                if ch is None:
                    ch = wo_loads(pblk)
                for jt in range(4):
                    po = pproj.tile([128, 512], F32, name="ps", tag="ps")
                    for ec in range(8):
                        nc.tensor.matmul(
                            po[:, :],
                            wo[ec][:, jt * 128:(jt + 1) * 128],
                            ch[ec][:, :],
                            start=(ec == 0), stop=(ec == 7))
                    ot = osb.tile([128, 512], F32, name="ot", tag="ot")
                    nc.vector.tensor_copy(ot[:, :], po[:, :])
                    nc.sync.dma_start(out[jt * 128:(jt + 1) * 128, psl],
                                      ot[:, :])

            def wo_loads(pblk):
                ch = []
                for ec in range(8):
                    r, m = ec // 4, ec % 4
                    gt = agp.tile([128, 512], BF, name="gt", tag="gt")
                    nc.sync.dma_start(
                        gt[:, :], ag_out[pblk][r, m * 128:(m + 1) * 128, :])
                    ch.append(gt)
                return ch

            xcache = {}
            A_EC, B_EC = [0, 1, 4, 5], [2, 3, 6, 7]
            ag_ch = {}
            wo_loaded = {}

            def load_x(b_):
                sl_ = slice(b_ * 512, (b_ + 1) * 512)
                chunks = []
                for c in range(8):
                    xb = xbp.tile([128, 512], BF, name="xb", tag="xb")
                    nc.sync.dma_start(xb[:, :],
                                      xT[c * 128:(c + 1) * 128, sl_])
                    chunks.append(xb)
                xcache[b_] = chunks

            # startup order: x block 0 first, then small consts, then
            # q weights (needed first), then the rest.
            load_x(0)
            for wdram, wtiles in ((wkT, wk), (wqT, wq), (wvT, wv),
                                  (woT, wo)):
                for c in range(8):
                    nc.sync.dma_start(wtiles[c][:, :],
                                      wdram[c * 128:(c + 1) * 128, :])
                if wdram is wkT:
                    # rope tables after k weights, before q weights
                    nc.sync.dma_start(cos_sb[:, :], cosT[:, :])
                    nc.sync.dma_start(sin_sb[:, :], sinT[:, :])
                    for d4 in range(2):
                        nc.sync.dma_start(mask_sb[d4][:, :], mask01[d4])

            for blk in range(NBLK):
                sl = slice(blk * 512, (blk + 1) * 512)
                xb_chunks = xcache.pop(blk)
                if blk + 1 < NBLK:
                    load_x(blk + 1)
                # k before q within each e-tile pair so the first heads'
                # QK (needs both) unblocks as early as possible
                for et in range(4):
                    for wtiles, dstT in ((wk, kT), (wq, qT)):
                        ps = pproj.tile([128, 512], F32, name="ps", tag="ps")
                        for c in range(8):
                            nc.tensor.matmul(
                                ps[:, :],
                                wtiles[c][:, et * 128:(et + 1) * 128],
                                xb_chunks[c][:, :],
                                start=(c == 0), stop=(c == 7))
                        # RoPE in bf16 (DVE 2x mode):
                        # dst = qb*cos + swap32(qb)*sin, with the 32-row
                        # partner swap folded into the t1 muls' input APs
                        qb = rpool.tile([128, 512], BF, name="qb", tag="qb")
                        nc.vector.tensor_copy(qb[:, :], ps[:, :])
                        t1 = rpool.tile([128, 512], BF, name="t1", tag="t1")
                        # sin_sb rows are pre-swapped on the host so both
                        # inputs share a base partition; only the output
                        # lands in the partner 32-row block.
                        for a, b_ in ((0, 32), (32, 0), (64, 96), (96, 64)):
                            nc.vector.tensor_mul(t1[a:a + 32, :],
                                                 qb[b_:b_ + 32, :],
                                                 sin_sb[b_:b_ + 32, sl])
                        t2 = rpool.tile([128, 512], BF, name="t2", tag="t2")
                        nc.vector.tensor_mul(t2[:, :], qb[:, :],
                                             cos_sb[:, sl])
                        nc.vector.tensor_add(dstT[et][:, sl], t2[:, :],
                                             t1[:, :])
                for st in range(4):
                    ti = blk * 4 + st
                    psv = pproj.tile([128, 512], F32, name="ps", tag="ps")
                    for c in range(8):
                        nc.tensor.matmul(
                            psv[:, :],
                            xb_chunks[c][:, st * 128:(st + 1) * 128],
                            wv[c][:, :],
                            start=(c == 0), stop=(c == 7))
                    nc.vector.tensor_copy(
                        vS[ti][:, :].rearrange("p (h c) -> p h c",
                                               c=HD + 1)[:, :, 0:HD],
                        psv[:, :].rearrange("p (h c) -> p h c", c=HD))
                    nc.vector.memset(
                        vS[ti][:, :].rearrange("p (h c) -> p h c",
                                               c=HD + 1)[:, :, HD:HD + 1],
                        1.0)

                # ---- attention for query-block blk, all heads ----
                bi = blk
                npair = 2 * bi + 2
                isl = slice(bi * 512, (bi + 1) * 512)
                for h in range(HPC):
                    ti, off = h // 2, (h % 2) * 64
                    oa = pav.tile([65, 512], F32, name="oa", tag="oa")
                    for jp in range(npair):
                        sc = psc.tile([128, 1024], F32, name="sc", tag="sc")
                        dp = jp - 2 * bi
                        # For the outermost diagonal pair (d=2,3) the
                        # causal-valid region is only the top 256/128
                        # query columns: narrow QK/exp/mask/AV to that
                        # rectangle.  (d=0,1 stay full width so the sc
                        # tile is never read where unwritten.)
                        los = [0, 0]
                        if dp == 1:
                            los = [256, 384]
                        for half in range(2):
                            jt = 2 * jp + half
                            lo = los[half]
                            nc.tensor.matmul(
                                sc[:, half * 512 + lo:(half + 1) * 512],
                                kT[ti][off:off + 64,
                                       jt * 128:(jt + 1) * 128],
                                qT[ti][off:off + 64,
                                       bi * 512 + lo:(bi + 1) * 512],
                                start=True, stop=True)
                        pt = ppool.tile([128, 1024], BF, name="pt", tag="pt")
                        if dp == 1:
                            for half in range(2):
                                lo = half * 512 + los[half]
                                hi = (half + 1) * 512
                                nc.scalar.activation(pt[:, lo:hi],
                                                     sc[:, lo:hi], AF.Exp,
                                                     scale=0.125)
                                nc.vector.tensor_mul(pt[:, lo:hi],
                                                     pt[:, lo:hi],
                                                     mask_sb[1][:, lo:hi])
                        else:
                            nc.scalar.activation(pt[:, :], sc[:, :], AF.Exp,
                                                 scale=0.125)
                            if dp == 0:
                                nc.vector.tensor_mul(pt[:, :], pt[:, :],
                                                     mask_sb[0][:, :])
                        for half in range(2):
                            jt = 2 * jp + half
                            lo = los[half]
                            nc.tensor.matmul(
                                oa[:, lo:512],
                                vS[jt][:, h * (HD + 1):(h + 1) * (HD + 1)],
                                pt[:, half * 512 + lo:(half + 1) * 512],
                                start=(jt == 0), stop=(jt == 2 * npair - 1))
                    # Copy raw output + denominator out of PSUM first so
                    # the oa slot recycles without waiting for the full
                    # cross-engine normalize chain.
                    rc = rrpool.tile([1, 512], F32, name="rc", tag="rc")
                    nc.vector.reciprocal(rc[:, :], oa[64:65, :])
                    raw = rrpool.tile([64, 512], F32, name="raw", tag="raw")
                    nc.vector.tensor_copy(raw[:, :], oa[0:64, :])
                    bcb = rrpool.tile([64, 512], F32, name="bcb", tag="bcb")
                    nc.gpsimd.partition_broadcast(bcb[:, :], rc[0:1, :])
                    nc.vector.scalar_tensor_tensor(
                        attnT[ti][off:off + 64, isl], raw[:, :], 1.0,
                        bcb[:, :], OP.mult, OP.mult)
                    # last block: exchange each head-half as soon as its
                    # attention completes, so only the second (quarter-
                    # size) AllGather is exposed at the end.
                    if blk == NBLK - 1 and h in (3, 7):
                        half = h // 4
                        for ti2 in range(2):
                            nc.sync.dma_start(
                                ag_in_l[half][ti2 * 128:(ti2 + 1) * 128, :],
                                attnT[half * 2 + ti2][:, isl])
                        nc.gpsimd.collective_compute(
                            "AllGather", OP.bypass, replica_groups=RG,
                            ins=[ag_in_l[half][:, :].opt()],
                            outs=[ag_out_l[half][:, :, :].opt()])
                        if half == 0:
                            # W_o for blocks 1/2 (collectives long done):
                            # their DMA loads must precede the second
                            # half's ag_in on the in-order queue, and
                            # (matmuls are emitted after the attention
                            # loop so the in-order PE queue reaches them
                            # during the final AllGather's flight)
                            for p_ in range(3):
                                wo_loaded[p_] = wo_loads(p_)
                            # final-block A-half gather loads early too
                            for ec in A_EC:
                                r, m = ec // 4, ec % 4
                                srcb = ag_out_l[m // 2]
                                gt = agp.tile([128, 512], BF, name="gt",
                                              tag="gt")
                                nc.sync.dma_start(
                                    gt[:, :],
                                    srcb[r, (m % 2) * 128:
                                         (m % 2 + 1) * 128, :])
                                ag_ch[ec] = gt

                if blk < NBLK - 1:
                    # AllGather for column-block blk; its W_o stage is
                    # emitted at the end of the next iteration.
                    for ti4 in range(4):
                        nc.sync.dma_start(
                            ag_in[blk][ti4 * 128:(ti4 + 1) * 128, :],
                            attnT[ti4][:, isl])
                    nc.gpsimd.collective_compute(
                        "AllGather", OP.bypass, replica_groups=RG,
                        ins=[ag_in[blk][:, :].opt()],
                        outs=[ag_out[blk][:, :, :].opt()])



            # W_o matmuls for blocks 0-2: PE reaches these right after
            # the last attention head, covering the final AllGather.
            wo_stage(0)
            wo_stage(1)
            wo_stage(2)

            # Final block's W_o, A/B-half software-pipelined: A halves
            # (loads emitted at h==3) need only the first AllGather; the
            # exposed wait on the last AllGather shrinks to the B halves.
            psl = slice((NBLK - 1) * 512, NBLK * 512)
            for ec in B_EC:
                r, m = ec // 4, ec % 4
                srcb = ag_out_l[m // 2]
                gt = agp.tile([128, 512], BF, name="gt", tag="gt")
                nc.sync.dma_start(
                    gt[:, :], srcb[r, (m % 2) * 128:(m % 2 + 1) * 128, :])
                ag_ch[ec] = gt
            po_t = {}

            def wo_half(jt, ecs, start, stop):
                if jt not in po_t:
                    po_t[jt] = pproj.tile([128, 512], F32, name="ps",
                                          tag="ps")
                for idx, ec in enumerate(ecs):
                    nc.tensor.matmul(
                        po_t[jt][:, :],
                        wo[ec][:, jt * 128:(jt + 1) * 128],
                        ag_ch[ec][:, :],
                        start=start and idx == 0,
                        stop=stop and idx == len(ecs) - 1)
                if stop:
                    ot = osb.tile([128, 512], F32, name="ot", tag="ot")
                    nc.vector.tensor_copy(ot[:, :], po_t[jt][:, :])
                    nc.sync.dma_start(out[jt * 128:(jt + 1) * 128, psl],
                                      ot[:, :])

            wo_half(0, A_EC, True, False)
            wo_half(1, A_EC, True, False)
            wo_half(0, B_EC, False, True)
            wo_half(2, A_EC, True, False)
            wo_half(1, B_EC, False, True)
            wo_half(3, A_EC, True, False)
            wo_half(2, B_EC, False, True)
            wo_half(3, B_EC, False, True)
    nc.finalize()
    return nc


def _host_prep(x, W_q, W_k, W_v, W_o, mask):
    causal = np.triu(np.ones((S, S), dtype=bool), k=1)
    m = np.asarray(mask)
    assert m.shape == (B, S, S) and all(
        np.array_equal(m[b], causal) for b in range(B)), \
        "kernel is specialized for the causal mask"

    perm = np.concatenate([np.arange(0, HD, 2), np.arange(1, HD, 2)])
    permD = (np.arange(H)[:, None] * HD + perm[None, :]).reshape(-1)
    Wq_p = np.asarray(W_q)[permD]
    Wk_p = np.asarray(W_k)[permD]

    inv = 1.0 / (10000.0 ** (np.arange(0, HD, 2, dtype=np.float64) / HD))
    t = np.arange(S, dtype=np.float64)
    emb = np.concatenate([t[:, None] * inv[None, :]] * 2, axis=1)  # [S, 64]
    cosF = np.cos(emb).T[perm]                       # [64, S]
    sinF = np.sin(emb).T[perm]
    sgn = np.concatenate([-np.ones(32), np.ones(32)])[:, None]
    import ml_dtypes
    bf16 = ml_dtypes.bfloat16
    cos128 = np.ascontiguousarray(np.tile(cosF, (2, 1)).astype(bf16))
    sin128 = np.tile(sinF * sgn, (2, 1))
    swap = np.concatenate([np.arange(32, 64), np.arange(0, 32),
                           np.arange(96, 128), np.arange(64, 96)])
    sin128 = np.ascontiguousarray(sin128[swap].astype(bf16))

    r = np.arange(128)[:, None]
    c = np.arange(512)[None, :]
    b4 = [np.where(d4 * 128 + r > c, 0.0, 1.0).astype(bf16)
          for d4 in range(4)]
    mask_np = np.stack([np.concatenate([b4[0], b4[1]], axis=1),
                        np.concatenate([b4[2], b4[3]], axis=1)])

    in_maps = []
    for core in range(8):
        b, hg = core // 2, core % 2
        rs = slice(hg * E, (hg + 1) * E)
        in_maps.append({
            "xT": np.ascontiguousarray(np.asarray(x)[b].T.astype(bf16)),
            "wqT": np.ascontiguousarray(Wq_p[rs].T.astype(bf16)),
            "wkT": np.ascontiguousarray(Wk_p[rs].T.astype(bf16)),
            "wvT": np.ascontiguousarray(np.asarray(W_v)[rs].T.astype(bf16)),
            "woT": np.ascontiguousarray(np.asarray(W_o)[rs].T.astype(bf16)),
            "cosT": cos128,
            "sinT": sin128,
            "mask01": mask_np,
        })
    return in_maps


def kernel(x, W_q, W_k, W_v, W_o, mask, _trace=False):
    from concourse.bass_utils import run_bass_kernel_spmd

    if "nc" not in _CACHE:
        _CACHE["nc"] = _build_nc()
    nc = _CACHE["nc"]
    in_maps = _host_prep(x, W_q, W_k, W_v, W_o, mask)
    res = run_bass_kernel_spmd(nc, in_maps, core_ids=list(range(8)),
                               trace=_trace)
    _CACHE["last_result"] = res
    full = np.empty((B, S, D), dtype=np.float32)
    for core in range(8):
        b, hg = core // 2, core % 2
        full[b, :, hg * E:(hg + 1) * E] = res.results[core]["out"].T
    return full
